# revision 44
# baseline (speedup 1.0000x reference)
"""Causal multi-head attention with RoPE on 8 Trainium2 NeuronCores.

Tensor-parallel over heads: each core owns 2 of the 16 heads (its slice of
qkv_w rows and o_w columns), computes attention + its partial o_proj for
both batch elements, and the host sums the 8 partial outputs (the
"all-reduce").

Device layout choices (see module builder):
  - q/k computed feature-major [dk, tokens] so scores need no transposes
  - scores computed transposed [k, q]; softmax denominator = ones-matmul
    partition reduction; no max-subtraction (scores are bounded, exp is
    safe in fp32)
  - RoPE via even/odd row-permuted projection weights + a DMA partition
    half-swap + 3 full-width vector ops
  - v kept token-major so attn@v consumes exp(scores) directly
  - all big matmuls in bf16 (1 cycle/row); accumulation in fp32 PSUM
"""

import math
from contextlib import ExitStack

import numpy as np
import ml_dtypes

D_MODEL = 2048
NUM_HEADS = 16
HEAD_DIM = 128
THETA = 10000.0
B = 2
S = 2048
NCORES = 8
HPC = NUM_HEADS // NCORES  # heads per core
F = HPC * HEAD_DIM  # q/k/v features per core

BF16 = ml_dtypes.bfloat16


def build_module(D=D_MODEL, S_=S, B_=B):
    import concourse.mybir as mybir
    import concourse.tile as tile
    from concourse import bacc
    from concourse.bass import ts, ds

    f32 = mybir.dt.float32
    bf16 = mybir.dt.bfloat16
    f32r = mybir.dt.float32r
    Exp = mybir.ActivationFunctionType.Exp

    T = B_ * S_
    DC = D // 128  # contraction chunks
    TG = 512  # token group for qkv projection
    NTG = T // TG
    GPB = S_ // TG  # t-groups per batch
    NQG = S_ // 512  # q-groups per batch
    NVC = T // 128  # v token chunks
    KPB = S_ // 128  # k tiles per batch
    scale = 1.0 / math.sqrt(HEAD_DIM)

    nc = bacc.Bacc("TRN2", target_bir_lowering=False, debug=False)

    xt = nc.dram_tensor("xt", [D, T], bf16, kind="ExternalInput")[:]
    wq = nc.dram_tensor("wq", [D, F], bf16, kind="ExternalInput")[:]
    wk = nc.dram_tensor("wk", [D, F], bf16, kind="ExternalInput")[:]
    wv = nc.dram_tensor("wv", [D, F], bf16, kind="ExternalInput")[:]
    wo = nc.dram_tensor("wo", [F, D], bf16, kind="ExternalInput")[:]
    cs2 = nc.dram_tensor("cs2", [128, S_], f32, kind="ExternalInput")[:]
    sn2 = nc.dram_tensor("sn2", [128, S_], f32, kind="ExternalInput")[:]
    mk = nc.dram_tensor("mk", [128, 4 * 512], bf16, kind="ExternalInput")[:]
    y = nc.dram_tensor("y", [T, D], f32, kind="ExternalOutput")[:]

    with tile.TileContext(nc) as tc:
        with ExitStack() as ctx:
            const = ctx.enter_context(tc.tile_pool(name="const", bufs=1))
            xpool = ctx.enter_context(tc.tile_pool(name="xpool", bufs=2))
            store = ctx.enter_context(tc.tile_pool(name="store", bufs=1))
            rope = ctx.enter_context(tc.tile_pool(name="rope", bufs=2))
            epool = ctx.enter_context(tc.tile_pool(name="epool", bufs=6))
            spool = ctx.enter_context(tc.tile_pool(name="spool", bufs=2))
            psum = ctx.enter_context(tc.tile_pool(name="psum", bufs=3, space="PSUM"))
            psgen = ctx.enter_context(tc.tile_pool(name="psgen", bufs=2, space="PSUM"))
            psacc = ctx.enter_context(tc.tile_pool(name="psacc", bufs=2, space="PSUM"))
            psred = ctx.enter_context(tc.tile_pool(name="psred", bufs=1, space="PSUM"))

            # ---- constants ----
            # DMA order matters at startup: the first QKV f-tile only needs
            # wq + xg(0); put those first on the queue. xg(0) itself is
            # emitted by the first emit_xg call below.
            wq_sb = const.tile([128, DC, F], bf16, tag="wq")
            nc.sync.dma_start(out=wq_sb[:], in_=wq.rearrange("(o p) f -> p o f", p=128))
            wk_sb = const.tile([128, DC, F], bf16, tag="wk")
            wv_sb = const.tile([128, DC, F], bf16, tag="wv")
            wo_sb = const.tile([128, HPC, D], bf16, tag="wo")
            cs2_sb = const.tile([128, S_], f32, tag="cs2")
            sn2_sb = const.tile([128, S_], f32, tag="sn2")
            mk_sb = const.tile([128, 4, 512], bf16, tag="mk")
            ones_sb = const.tile([128, 128], bf16, tag="ones")

            def emit_late_consts():
                # rope tables for the first token block only, then the rest
                nc.sync.dma_start(out=cs2_sb[:, 0:TG], in_=cs2[:, 0:TG])
                nc.sync.dma_start(out=sn2_sb[:, 0:TG], in_=sn2[:, 0:TG])
                nc.sync.dma_start(
                    out=wk_sb[:], in_=wk.rearrange("(o p) f -> p o f", p=128)
                )
                if GPB > 1:
                    emit_xg(1)
                    nc.sync.dma_start(out=cs2_sb[:, TG:], in_=cs2[:, TG:])
                    nc.sync.dma_start(out=sn2_sb[:, TG:], in_=sn2[:, TG:])
                nc.sync.dma_start(
                    out=wv_sb[:], in_=wv.rearrange("(o p) f -> p o f", p=128)
                )
                nc.vector.memset(ones_sb[:], 1.0)

            def emit_attn_consts():
                nc.sync.dma_start(
                    out=mk_sb[:], in_=mk.rearrange("p (m q) -> p m q", m=4)
                )
                nc.sync.dma_start(
                    out=wo_sb[:], in_=wo.rearrange("(o p) e -> p o e", p=128)
                )

            # ---- persistent stores ----
            q_sb = store.tile([128, HPC, T], bf16, tag="q")  # [dk, ht, tok]
            k_sb = store.tile([128, HPC, T], bf16, tag="k")
            v_sb = store.tile([128, NVC, F], bf16, tag="v")  # [tok128, chunk, f]
            ao_sb = store.tile([128, HPC, T], bf16, tag="ao")  # [dk, ht, tok]

            xt_r = xt.rearrange("(o p) t -> p o t", p=128)

            qk_jobs = [(wq_sb, q_sb, 0), (wq_sb, q_sb, 1), (wk_sb, k_sb, 0), (wk_sb, k_sb, 1)]
            xg_tiles = {}

            def emit_xg(g, split=1):
                xg = xpool.tile([128, DC, TG], bf16, tag="xg", name=f"xg_{g}")
                # split lets the first matmul start after the first chunk lands
                step = DC // split
                for s in range(split):
                    nc.sync.dma_start(
                        out=xg[:, ts(s, step), :],
                        in_=xt_r[:, ts(s, step), ts(g, TG)],
                    )
                xg_tiles[g] = xg

            def emit_qk_ftile(g, job):
                w_sb, qk_store, ht = job
                xg = xg_tiles[g]
                pos_sl = ds((g % GPB) * TG, TG)
                ps = psgen.tile([128, TG], f32, tag="gen")
                for dc in range(DC):
                    nc.tensor.matmul(
                        ps[:],
                        lhsT=w_sb[:, dc, ts(ht, 128)],
                        rhs=xg[:, dc, :],
                        start=(dc == 0),
                        stop=(dc == DC - 1),
                    )
                # rope: rot = raw*cos2 + halfswap(raw*sin2_pre)
                t0 = rope.tile([128, TG], f32, tag="t0")
                nc.vector.tensor_mul(out=t0[:], in0=ps[:], in1=cs2_sb[:, pos_sl])
                t1s = rope.tile([128, TG], f32, tag="t1s")
                nc.vector.tensor_mul(out=t1s[:], in0=ps[:], in1=sn2_sb[:, pos_sl])
                # scalar (ACT) HWDGE ring: keeps these latency-critical swaps
                # out from behind the 2MB xg loads on the sync ring
                t1w = rope.tile([128, TG], f32, tag="t1w")
                nc.scalar.dma_start(out=t1w[0:64, :], in_=t1s[64:128, :])
                nc.scalar.dma_start(out=t1w[64:128, :], in_=t1s[0:64, :])
                nc.vector.tensor_add(
                    out=qk_store[:, ht, ts(g, TG)], in0=t0[:], in1=t1w[:]
                )

            def emit_v_sub(g, tsub):
                xg = xg_tiles[g]
                psv = psgen.tile([128, F], f32, tag="gen")
                for dc in range(DC):
                    nc.tensor.matmul(
                        psv[:],
                        lhsT=xg[:, dc, ts(tsub, 128)],
                        rhs=wv_sb[:, dc, :],
                        start=(dc == 0),
                        stop=(dc == DC - 1),
                    )
                nc.scalar.copy(out=v_sb[:, g * (TG // 128) + tsub, :], in_=psv[:])

            def emit_qkv_group(g):
                emit_xg(g)
                for job in qk_jobs:
                    emit_qk_ftile(g, job)
                for tsub in range(TG // 128):
                    emit_v_sub(g, tsub)

            def emit_oproj_eg(tt, eg, ystate):
                if eg == 0:
                    ystate[tt] = rope.tile([128, D], f32, tag="ysb", name=f"ysb_{tt}")
                y_sb = ystate[tt]
                yp = psgen.tile([128, 512], f32, tag="gen")
                for ht in range(HPC):
                    nc.tensor.matmul(
                        yp[:],
                        lhsT=ao_sb[:, ht, ts(tt, 128)],
                        rhs=wo_sb[:, ht, ts(eg, 512)],
                        start=(ht == 0),
                        stop=(ht == HPC - 1),
                    )
                if eg % 2 == 0:
                    nc.vector.tensor_copy(out=y_sb[:, ts(eg, 512)], in_=yp[:])
                else:
                    nc.scalar.copy(out=y_sb[:, ts(eg, 512)], in_=yp[:])
                if eg == D // 512 - 1:
                    nc.sync.dma_start(out=y[ts(tt, 128), :], in_=y_sb[:])
                    del ystate[tt]

            ystate = {}

            def emit_oproj_tile(tt):
                for eg in range(D // 512):
                    emit_oproj_eg(tt, eg, ystate)

            # filler queue: zero-dependency emission units pumped into PE
            # stalls of the attention dependency chain
            from collections import deque

            filler = deque()

            def pump(n=1):
                for _ in range(n):
                    if filler:
                        filler.popleft()()

            def drain():
                while filler:
                    filler.popleft()()

            LOOKAHEAD = 2

            def emit_attention_batch(b, post_group_hook=None):
                pending = [None]
                total_steps = HPC * sum(4 * m + 4 for m in range(NQG))
                state = {"done": 0, "acc": 0.0}

                def paced_pump():
                    # spread the filler queue evenly over the remaining
                    # attention steps so PE never runs a bare chain
                    state["done"] += 1
                    remaining = max(1, total_steps - state["done"])
                    state["acc"] += len(filler) / remaining
                    n = int(state["acc"])
                    if n:
                        state["acc"] -= n
                        pump(n)

                def do_pending():
                    if pending[0] is not None:
                        pending[0]()
                        pending[0] = None

                for m in range(NQG):
                    for ht in range(HPC):
                        qv = q_sb[:, ht, ds(b * S_, S_)]
                        kv = k_sb[:, ht, ds(b * S_, S_)]
                        nk = 4 * m + 4
                        ao = psacc.tile([128, 512], f32, tag="acc")
                        sacc = spool.tile([128, 512], f32, tag="sacc")
                        eTs = {}
                        steps = 0

                        def emit_scores(j, m=m, ao=ao, sacc=sacc, eTs=eTs, qv=qv, kv=kv):
                            p = j - 4 * m  # >= 0 on diagonal superblock
                            col0 = max(0, p * 128)
                            ncol = 512 - col0
                            s_ps = psum.tile([128, 512], f32, tag="work")
                            nc.tensor.matmul(
                                s_ps[:, col0:],
                                lhsT=kv[:, ts(j, 128)],
                                rhs=qv[:, ds(m * 512 + col0, ncol)],
                                start=True,
                                stop=True,
                            )
                            eT = epool.tile([128, 512], bf16, tag="eT")
                            nc.scalar.activation(
                                out=eT[:, col0:], in_=s_ps[:, col0:], func=Exp, scale=scale
                            )
                            if p >= 0:
                                nc.vector.tensor_mul(
                                    out=eT[:, col0:],
                                    in0=eT[:, col0:],
                                    in1=mk_sb[:, p, col0:],
                                )
                            if j == 0:
                                nc.vector.tensor_copy(out=sacc[:], in_=eT[:])
                            else:
                                nc.vector.tensor_add(
                                    out=sacc[:, col0:],
                                    in0=sacc[:, col0:],
                                    in1=eT[:, col0:],
                                )
                            eTs[j] = (eT, col0)

                        def emit_attnv(j, m=m, ao=ao, eTs=eTs, b=b, ht=ht, nk=nk):
                            eT, col0 = eTs.pop(j)
                            nc.tensor.matmul(
                                ao[:, col0:],
                                lhsT=v_sb[:, b * KPB + j, ts(ht, 128)],
                                rhs=eT[:, col0:],
                                start=(j == 0),
                                stop=(j == nk - 1),
                            )

                        for j in range(nk):
                            emit_scores(j)
                            if j == 1:
                                do_pending()
                            if j >= LOOKAHEAD:
                                emit_attnv(j - LOOKAHEAD)
                            steps += 1
                            paced_pump()
                        for j in range(max(0, nk - LOOKAHEAD), nk):
                            emit_attnv(j)

                        def finalize(b=b, ht=ht, m=m, ao=ao, sacc=sacc):
                            sacc_b = epool.tile([128, 512], bf16, tag="saccb")
                            nc.vector.tensor_copy(out=sacc_b[:], in_=sacc[:])
                            rps = psred.tile([128, 512], f32, tag="red")
                            nc.tensor.matmul(
                                rps[:],
                                lhsT=ones_sb[:],
                                rhs=sacc_b[:],
                                start=True,
                                stop=True,
                            )
                            rsb = spool.tile([128, 512], f32, tag="rsb")
                            nc.vector.reciprocal_approx_fast(out=rsb[:], in_=rps[:])
                            nc.vector.tensor_mul(
                                out=ao_sb[:, ht, ds(b * S_ + m * 512, 512)],
                                in0=ao[:],
                                in1=rsb[:],
                            )
                            if ht == HPC - 1 and post_group_hook is not None:
                                post_group_hook(m)

                        pending[0] = finalize
                do_pending()

            # ---- program order ----
            emit_xg(0, split=4)
            emit_late_consts()
            for g in range(GPB):  # batch-0 qkv, dense; prefetch next xg
                if 1 < g + 1 < GPB:
                    emit_xg(g + 1)
                for job in qk_jobs:
                    emit_qk_ftile(g, job)
                for tsub in range(TG // 128):
                    emit_v_sub(g, tsub)
            emit_attn_consts()
            if B_ > 1:
                for g in range(GPB, NTG):  # batch-1 qkv as filler
                    filler.append((lambda g=g: emit_xg(g)))
                    for job in qk_jobs:
                        filler.append(lambda g=g, job=job: emit_qk_ftile(g, job))
                    for tsub in range(TG // 128):
                        filler.append(lambda g=g, tsub=tsub: emit_v_sub(g, tsub))

            def oproj_hook(b):
                def hook(m):
                    # q-group m fully normalized -> its four 128-token oproj
                    # tiles are ready; queue them as filler
                    base = b * (T // 256)
                    for tt in range(base + 4 * m, base + 4 * m + 4):
                        filler.append(lambda tt=tt: emit_oproj_tile(tt))

                return hook

            emit_attention_batch(0, post_group_hook=oproj_hook(0) if B_ > 1 else None)
            drain()  # leftovers of batch-1 qkv / batch-0 oproj
            if B_ > 1:
                emit_attention_batch(1, post_group_hook=oproj_hook(1))
                drain()
            else:
                for tt in range(T // 128):
                    emit_oproj_tile(tt)

    nc.compile()
    return nc


def _rope_tables(token_positions, S_):
    pos = np.asarray(token_positions).astype(np.float32)
    dim_id = np.arange(0, HEAD_DIM, 2, dtype=np.float32)
    inv_freq = np.power(np.float32(THETA), dim_id / np.float32(HEAD_DIM)).astype(
        np.float32
    )
    ang = (pos[None, :] / inv_freq[:, None]).astype(np.float32)  # [64, S]
    cos = np.cos(ang).astype(np.float32)
    sin = np.sin(ang).astype(np.float32)
    cs2 = np.concatenate([cos, cos], axis=0)  # [128, S]
    # pre-multiply sign layout: top half (x0 rows) gets +sin (feeds r1 after
    # the half-swap), bottom half (x1 rows) gets -sin (feeds r0)
    sn2 = np.concatenate([sin, -sin], axis=0)
    return np.ascontiguousarray(cs2), np.ascontiguousarray(sn2)


def _masks():
    mk = np.zeros((128, 4, 512), dtype=np.float32)
    kl = np.arange(128)[:, None]
    ql = np.arange(512)[None, :]
    for p in range(4):
        mk[:, p, :] = ((p * 128 + kl) <= ql).astype(np.float32)
    return np.ascontiguousarray(mk.reshape(128, 4 * 512).astype(BF16))


def _perm(n_heads):
    # within each 128-feature head block: evens then odds
    p = []
    for h in range(n_heads):
        base = h * HEAD_DIM
        p.extend(range(base, base + HEAD_DIM, 2))
        p.extend(range(base + 1, base + HEAD_DIM, 2))
    return np.array(p, dtype=np.int64)


def prepare_in_maps(x, token_positions, qkv_w, o_w, D=D_MODEL, S_=S, B_=B, ncores=NCORES):
    T = B_ * S_
    x = np.asarray(x, dtype=np.float32)
    qkv_w = np.asarray(qkv_w, dtype=np.float32)
    o_w = np.asarray(o_w, dtype=np.float32)

    xt = np.ascontiguousarray(x.reshape(T, D).T).astype(BF16)  # [D, T]
    cs2, sn2 = _rope_tables(token_positions, S_)
    mk = _masks()
    perm = _perm(HPC)

    in_maps = []
    for c in range(ncores):
        r0 = c * F
        qrows = qkv_w[r0 : r0 + F]
        krows = qkv_w[D + r0 : D + r0 + F]
        vrows = qkv_w[2 * D + r0 : 2 * D + r0 + F]
        wq_c = np.ascontiguousarray(qrows[perm].T).astype(BF16)  # [D, F]
        wk_c = np.ascontiguousarray(krows[perm].T).astype(BF16)
        wv_c = np.ascontiguousarray(vrows.T).astype(BF16)
        wo_c = np.ascontiguousarray(o_w[:, r0 : r0 + F].T).astype(BF16)  # [F, D]
        in_maps.append(
            {
                "xt": xt,
                "wq": wq_c,
                "wk": wk_c,
                "wv": wv_c,
                "wo": wo_c,
                "cs2": cs2,
                "sn2": sn2,
                "mk": mk,
            }
        )
    return in_maps


_CACHE = {}


def kernel_with_results(x, token_positions, qkv_w, o_w, trace=False, **kw):
    from concourse.bass_utils import run_bass_kernel_spmd

    if "nc" not in _CACHE:
        _CACHE["nc"] = build_module()
    nc = _CACHE["nc"]

    in_maps = prepare_in_maps(x, token_positions, qkv_w, o_w)
    res = run_bass_kernel_spmd(
        nc, in_maps, core_ids=list(range(NCORES)), trace=trace, **kw
    )
    acc = np.zeros((B * S, D_MODEL), dtype=np.float64)
    for r in res.results:
        acc += r["y"].astype(np.float64)
    return acc.astype(np.float32).reshape(B, S, D_MODEL), res


def kernel(x, token_positions, qkv_w, o_w):
    out, _ = kernel_with_results(x, token_positions, qkv_w, o_w)
    return out


# revision 45
# speedup vs baseline: 1.0239x; 1.0239x over previous
"""Causal multi-head attention with RoPE on 8 Trainium2 NeuronCores.

Tensor-parallel over heads: each core owns 2 of the 16 heads (its slice of
qkv_w rows and o_w columns), computes attention + its partial o_proj for
both batch elements, and the host sums the 8 partial outputs (the
"all-reduce").

Device layout choices (see module builder):
  - q/k computed feature-major [dk, tokens] so scores need no transposes
  - scores computed transposed [k, q]; softmax denominator = ones-matmul
    partition reduction; no max-subtraction (scores are bounded, exp is
    safe in fp32)
  - RoPE via even/odd row-permuted projection weights + a DMA partition
    half-swap + 3 full-width vector ops
  - v kept token-major so attn@v consumes exp(scores) directly
  - all big matmuls in bf16 (1 cycle/row); accumulation in fp32 PSUM
"""

import math
from contextlib import ExitStack

import numpy as np
import ml_dtypes

D_MODEL = 2048
NUM_HEADS = 16
HEAD_DIM = 128
THETA = 10000.0
B = 2
S = 2048
NCORES = 8
HPC = NUM_HEADS // NCORES  # heads per core
F = HPC * HEAD_DIM  # q/k/v features per core

BF16 = ml_dtypes.bfloat16


def build_module(D=D_MODEL, S_=S, B_=B):
    import concourse.mybir as mybir
    import concourse.tile as tile
    from concourse import bacc
    from concourse.bass import ts, ds

    f32 = mybir.dt.float32
    bf16 = mybir.dt.bfloat16
    f32r = mybir.dt.float32r
    Exp = mybir.ActivationFunctionType.Exp

    T = B_ * S_
    DC = D // 128  # contraction chunks
    TG = 512  # token group for qkv projection
    NTG = T // TG
    GPB = S_ // TG  # t-groups per batch
    NQG = S_ // 512  # q-groups per batch
    NVC = T // 128  # v token chunks
    KPB = S_ // 128  # k tiles per batch
    scale = 1.0 / math.sqrt(HEAD_DIM)

    nc = bacc.Bacc("TRN2", target_bir_lowering=False, debug=False)

    xt = nc.dram_tensor("xt", [D, T], bf16, kind="ExternalInput")[:]
    wq = nc.dram_tensor("wq", [D, F], bf16, kind="ExternalInput")[:]
    wk = nc.dram_tensor("wk", [D, F], bf16, kind="ExternalInput")[:]
    wv = nc.dram_tensor("wv", [D, F], bf16, kind="ExternalInput")[:]
    wo = nc.dram_tensor("wo", [F, D], bf16, kind="ExternalInput")[:]
    cs2 = nc.dram_tensor("cs2", [128, S_], f32, kind="ExternalInput")[:]
    sn2 = nc.dram_tensor("sn2", [128, S_], f32, kind="ExternalInput")[:]
    mk = nc.dram_tensor("mk", [128, 4 * 512], bf16, kind="ExternalInput")[:]
    y = nc.dram_tensor("y", [T, D], f32, kind="ExternalOutput")[:]

    with tile.TileContext(nc) as tc:
        with ExitStack() as ctx:
            const = ctx.enter_context(tc.tile_pool(name="const", bufs=1))
            xpool = ctx.enter_context(tc.tile_pool(name="xpool", bufs=2))
            store = ctx.enter_context(tc.tile_pool(name="store", bufs=1))
            rope = ctx.enter_context(tc.tile_pool(name="rope", bufs=2))
            epool = ctx.enter_context(tc.tile_pool(name="epool", bufs=6))
            spool = ctx.enter_context(tc.tile_pool(name="spool", bufs=2))
            psum = ctx.enter_context(tc.tile_pool(name="psum", bufs=3, space="PSUM"))
            psgen = ctx.enter_context(tc.tile_pool(name="psgen", bufs=2, space="PSUM"))
            psacc = ctx.enter_context(tc.tile_pool(name="psacc", bufs=2, space="PSUM"))
            psred = ctx.enter_context(tc.tile_pool(name="psred", bufs=1, space="PSUM"))

            # ---- constants ----
            # DMA order matters at startup: the first QKV f-tile only needs
            # wq + xg(0); put those first on the queue. xg(0) itself is
            # emitted by the first emit_xg call below.
            wq_sb = const.tile([128, DC, F], bf16, tag="wq")
            nc.sync.dma_start(out=wq_sb[:], in_=wq.rearrange("(o p) f -> p o f", p=128))
            wk_sb = const.tile([128, DC, F], bf16, tag="wk")
            wv_sb = const.tile([128, DC, F], bf16, tag="wv")
            wo_sb = const.tile([128, HPC, D], bf16, tag="wo")
            cs2_sb = const.tile([128, S_], f32, tag="cs2")
            sn2_sb = const.tile([128, S_], f32, tag="sn2")
            mk_sb = const.tile([128, 4, 512], bf16, tag="mk")
            ones_sb = const.tile([128, 128], bf16, tag="ones")

            def emit_late_consts():
                # rope tables for the first token block only, then the rest
                # wk/wv ride the otherwise-idle scalar HWDGE ring so they
                # land in parallel with the sync ring's wq/xg/table loads
                nc.scalar.dma_start(
                    out=wk_sb[:], in_=wk.rearrange("(o p) f -> p o f", p=128)
                )
                nc.scalar.dma_start(
                    out=wv_sb[:], in_=wv.rearrange("(o p) f -> p o f", p=128)
                )
                nc.sync.dma_start(out=cs2_sb[:, 0:TG], in_=cs2[:, 0:TG])
                nc.sync.dma_start(out=sn2_sb[:, 0:TG], in_=sn2[:, 0:TG])
                if GPB > 1:
                    emit_xg(1)
                    nc.sync.dma_start(out=cs2_sb[:, TG:], in_=cs2[:, TG:])
                    nc.sync.dma_start(out=sn2_sb[:, TG:], in_=sn2[:, TG:])
                nc.vector.memset(ones_sb[:], 1.0)

            def emit_attn_consts():
                nc.sync.dma_start(
                    out=mk_sb[:], in_=mk.rearrange("p (m q) -> p m q", m=4)
                )
                nc.sync.dma_start(
                    out=wo_sb[:], in_=wo.rearrange("(o p) e -> p o e", p=128)
                )

            # ---- persistent stores ----
            q_sb = store.tile([128, HPC, T], bf16, tag="q")  # [dk, ht, tok]
            k_sb = store.tile([128, HPC, T], bf16, tag="k")
            v_sb = store.tile([128, NVC, F], bf16, tag="v")  # [tok128, chunk, f]
            ao_sb = store.tile([128, HPC, T], bf16, tag="ao")  # [dk, ht, tok]

            xt_r = xt.rearrange("(o p) t -> p o t", p=128)

            qk_jobs = [(wq_sb, q_sb, 0), (wq_sb, q_sb, 1), (wk_sb, k_sb, 0), (wk_sb, k_sb, 1)]
            xg_tiles = {}

            def emit_xg(g, split=1):
                xg = xpool.tile([128, DC, TG], bf16, tag="xg", name=f"xg_{g}")
                # split lets the first matmul start after the first chunk lands
                step = DC // split
                for s in range(split):
                    nc.sync.dma_start(
                        out=xg[:, ts(s, step), :],
                        in_=xt_r[:, ts(s, step), ts(g, TG)],
                    )
                xg_tiles[g] = xg

            def emit_qk_ftile(g, job):
                w_sb, qk_store, ht = job
                xg = xg_tiles[g]
                pos_sl = ds((g % GPB) * TG, TG)
                ps = psgen.tile([128, TG], f32, tag="gen")
                for dc in range(DC):
                    nc.tensor.matmul(
                        ps[:],
                        lhsT=w_sb[:, dc, ts(ht, 128)],
                        rhs=xg[:, dc, :],
                        start=(dc == 0),
                        stop=(dc == DC - 1),
                    )
                # rope: rot = raw*cos2 + halfswap(raw*sin2_pre)
                t0 = rope.tile([128, TG], f32, tag="t0")
                nc.vector.tensor_mul(out=t0[:], in0=ps[:], in1=cs2_sb[:, pos_sl])
                t1s = rope.tile([128, TG], f32, tag="t1s")
                nc.vector.tensor_mul(out=t1s[:], in0=ps[:], in1=sn2_sb[:, pos_sl])
                # scalar (ACT) HWDGE ring: keeps these latency-critical swaps
                # out from behind the 2MB xg loads on the sync ring
                t1w = rope.tile([128, TG], f32, tag="t1w")
                nc.scalar.dma_start(out=t1w[0:64, :], in_=t1s[64:128, :])
                nc.scalar.dma_start(out=t1w[64:128, :], in_=t1s[0:64, :])
                nc.vector.tensor_add(
                    out=qk_store[:, ht, ts(g, TG)], in0=t0[:], in1=t1w[:]
                )

            def emit_v_sub(g, tsub):
                xg = xg_tiles[g]
                psv = psgen.tile([128, F], f32, tag="gen")
                for dc in range(DC):
                    nc.tensor.matmul(
                        psv[:],
                        lhsT=xg[:, dc, ts(tsub, 128)],
                        rhs=wv_sb[:, dc, :],
                        start=(dc == 0),
                        stop=(dc == DC - 1),
                    )
                nc.scalar.copy(out=v_sb[:, g * (TG // 128) + tsub, :], in_=psv[:])

            def emit_qkv_group(g):
                emit_xg(g)
                for job in qk_jobs:
                    emit_qk_ftile(g, job)
                for tsub in range(TG // 128):
                    emit_v_sub(g, tsub)

            def emit_oproj_eg(tt, eg, ystate):
                if eg == 0:
                    ystate[tt] = rope.tile([128, D], f32, tag="ysb", name=f"ysb_{tt}")
                y_sb = ystate[tt]
                yp = psgen.tile([128, 512], f32, tag="gen")
                for ht in range(HPC):
                    nc.tensor.matmul(
                        yp[:],
                        lhsT=ao_sb[:, ht, ts(tt, 128)],
                        rhs=wo_sb[:, ht, ts(eg, 512)],
                        start=(ht == 0),
                        stop=(ht == HPC - 1),
                    )
                if eg % 2 == 0:
                    nc.vector.tensor_copy(out=y_sb[:, ts(eg, 512)], in_=yp[:])
                else:
                    nc.scalar.copy(out=y_sb[:, ts(eg, 512)], in_=yp[:])
                if eg == D // 512 - 1:
                    nc.sync.dma_start(out=y[ts(tt, 128), :], in_=y_sb[:])
                    del ystate[tt]

            ystate = {}

            def emit_oproj_tile(tt):
                for eg in range(D // 512):
                    emit_oproj_eg(tt, eg, ystate)

            # filler queue: zero-dependency emission units pumped into PE
            # stalls of the attention dependency chain
            from collections import deque

            filler = deque()

            def pump(n=1):
                for _ in range(n):
                    if filler:
                        filler.popleft()()

            def drain():
                while filler:
                    filler.popleft()()

            LOOKAHEAD = 2

            def emit_attention_batch(b, post_group_hook=None):
                pending = [None]
                total_steps = HPC * sum(4 * m + 4 for m in range(NQG))
                state = {"done": 0, "acc": 0.0}

                def paced_pump():
                    # spread the filler queue evenly over the remaining
                    # attention steps so PE never runs a bare chain
                    state["done"] += 1
                    remaining = max(1, total_steps - state["done"])
                    state["acc"] += len(filler) / remaining
                    n = int(state["acc"])
                    if n:
                        state["acc"] -= n
                        pump(n)

                def do_pending():
                    if pending[0] is not None:
                        pending[0]()
                        pending[0] = None

                for m in range(NQG):
                    for ht in range(HPC):
                        qv = q_sb[:, ht, ds(b * S_, S_)]
                        kv = k_sb[:, ht, ds(b * S_, S_)]
                        nk = 4 * m + 4
                        ao = psacc.tile([128, 512], f32, tag="acc")
                        sacc = spool.tile([128, 512], f32, tag="sacc")
                        eTs = {}
                        steps = 0

                        def emit_scores(j, m=m, ao=ao, sacc=sacc, eTs=eTs, qv=qv, kv=kv):
                            p = j - 4 * m  # >= 0 on diagonal superblock
                            col0 = max(0, p * 128)
                            ncol = 512 - col0
                            s_ps = psum.tile([128, 512], f32, tag="work")
                            nc.tensor.matmul(
                                s_ps[:, col0:],
                                lhsT=kv[:, ts(j, 128)],
                                rhs=qv[:, ds(m * 512 + col0, ncol)],
                                start=True,
                                stop=True,
                            )
                            eT = epool.tile([128, 512], bf16, tag="eT")
                            nc.scalar.activation(
                                out=eT[:, col0:], in_=s_ps[:, col0:], func=Exp, scale=scale
                            )
                            if p >= 0:
                                nc.vector.tensor_mul(
                                    out=eT[:, col0:],
                                    in0=eT[:, col0:],
                                    in1=mk_sb[:, p, col0:],
                                )
                            if j == 0:
                                nc.vector.tensor_copy(out=sacc[:], in_=eT[:])
                            else:
                                nc.vector.tensor_add(
                                    out=sacc[:, col0:],
                                    in0=sacc[:, col0:],
                                    in1=eT[:, col0:],
                                )
                            eTs[j] = (eT, col0)

                        def emit_attnv(j, m=m, ao=ao, eTs=eTs, b=b, ht=ht, nk=nk):
                            eT, col0 = eTs.pop(j)
                            nc.tensor.matmul(
                                ao[:, col0:],
                                lhsT=v_sb[:, b * KPB + j, ts(ht, 128)],
                                rhs=eT[:, col0:],
                                start=(j == 0),
                                stop=(j == nk - 1),
                            )

                        for j in range(nk):
                            emit_scores(j)
                            if j == 1:
                                do_pending()
                            if j >= LOOKAHEAD:
                                emit_attnv(j - LOOKAHEAD)
                            steps += 1
                            paced_pump()
                        for j in range(max(0, nk - LOOKAHEAD), nk):
                            emit_attnv(j)

                        def finalize(b=b, ht=ht, m=m, ao=ao, sacc=sacc):
                            sacc_b = epool.tile([128, 512], bf16, tag="saccb")
                            nc.vector.tensor_copy(out=sacc_b[:], in_=sacc[:])
                            rps = psred.tile([128, 512], f32, tag="red")
                            nc.tensor.matmul(
                                rps[:],
                                lhsT=ones_sb[:],
                                rhs=sacc_b[:],
                                start=True,
                                stop=True,
                            )
                            rsb = spool.tile([128, 512], f32, tag="rsb")
                            nc.vector.reciprocal_approx_fast(out=rsb[:], in_=rps[:])
                            nc.vector.tensor_mul(
                                out=ao_sb[:, ht, ds(b * S_ + m * 512, 512)],
                                in0=ao[:],
                                in1=rsb[:],
                            )
                            if ht == HPC - 1 and post_group_hook is not None:
                                post_group_hook(m)

                        pending[0] = finalize
                do_pending()

            # ---- program order ----
            emit_xg(0, split=4)
            emit_late_consts()
            for g in range(GPB):  # batch-0 qkv, dense; prefetch next xg
                if 1 < g + 1 < GPB:
                    emit_xg(g + 1)
                for job in qk_jobs:
                    emit_qk_ftile(g, job)
                for tsub in range(TG // 128):
                    emit_v_sub(g, tsub)
            emit_attn_consts()
            if B_ > 1:
                for g in range(GPB, NTG):  # batch-1 qkv as filler
                    filler.append((lambda g=g: emit_xg(g)))
                    for job in qk_jobs:
                        filler.append(lambda g=g, job=job: emit_qk_ftile(g, job))
                    for tsub in range(TG // 128):
                        filler.append(lambda g=g, tsub=tsub: emit_v_sub(g, tsub))

            def oproj_hook(b):
                def hook(m):
                    # q-group m fully normalized -> its four 128-token oproj
                    # tiles are ready; queue them as filler
                    base = b * (T // 256)
                    for tt in range(base + 4 * m, base + 4 * m + 4):
                        filler.append(lambda tt=tt: emit_oproj_tile(tt))

                return hook

            emit_attention_batch(0, post_group_hook=oproj_hook(0) if B_ > 1 else None)
            drain()  # leftovers of batch-1 qkv / batch-0 oproj
            if B_ > 1:
                emit_attention_batch(1, post_group_hook=oproj_hook(1))
                drain()
            else:
                for tt in range(T // 128):
                    emit_oproj_tile(tt)

    nc.compile()
    return nc


def _rope_tables(token_positions, S_):
    pos = np.asarray(token_positions).astype(np.float32)
    dim_id = np.arange(0, HEAD_DIM, 2, dtype=np.float32)
    inv_freq = np.power(np.float32(THETA), dim_id / np.float32(HEAD_DIM)).astype(
        np.float32
    )
    ang = (pos[None, :] / inv_freq[:, None]).astype(np.float32)  # [64, S]
    cos = np.cos(ang).astype(np.float32)
    sin = np.sin(ang).astype(np.float32)
    cs2 = np.concatenate([cos, cos], axis=0)  # [128, S]
    # pre-multiply sign layout: top half (x0 rows) gets +sin (feeds r1 after
    # the half-swap), bottom half (x1 rows) gets -sin (feeds r0)
    sn2 = np.concatenate([sin, -sin], axis=0)
    return np.ascontiguousarray(cs2), np.ascontiguousarray(sn2)


def _masks():
    mk = np.zeros((128, 4, 512), dtype=np.float32)
    kl = np.arange(128)[:, None]
    ql = np.arange(512)[None, :]
    for p in range(4):
        mk[:, p, :] = ((p * 128 + kl) <= ql).astype(np.float32)
    return np.ascontiguousarray(mk.reshape(128, 4 * 512).astype(BF16))


def _perm(n_heads):
    # within each 128-feature head block: evens then odds
    p = []
    for h in range(n_heads):
        base = h * HEAD_DIM
        p.extend(range(base, base + HEAD_DIM, 2))
        p.extend(range(base + 1, base + HEAD_DIM, 2))
    return np.array(p, dtype=np.int64)


def prepare_in_maps(x, token_positions, qkv_w, o_w, D=D_MODEL, S_=S, B_=B, ncores=NCORES):
    T = B_ * S_
    x = np.asarray(x, dtype=np.float32)
    qkv_w = np.asarray(qkv_w, dtype=np.float32)
    o_w = np.asarray(o_w, dtype=np.float32)

    xt = np.ascontiguousarray(x.reshape(T, D).T).astype(BF16)  # [D, T]
    cs2, sn2 = _rope_tables(token_positions, S_)
    mk = _masks()
    perm = _perm(HPC)

    in_maps = []
    for c in range(ncores):
        r0 = c * F
        qrows = qkv_w[r0 : r0 + F]
        krows = qkv_w[D + r0 : D + r0 + F]
        vrows = qkv_w[2 * D + r0 : 2 * D + r0 + F]
        wq_c = np.ascontiguousarray(qrows[perm].T).astype(BF16)  # [D, F]
        wk_c = np.ascontiguousarray(krows[perm].T).astype(BF16)
        wv_c = np.ascontiguousarray(vrows.T).astype(BF16)
        wo_c = np.ascontiguousarray(o_w[:, r0 : r0 + F].T).astype(BF16)  # [F, D]
        in_maps.append(
            {
                "xt": xt,
                "wq": wq_c,
                "wk": wk_c,
                "wv": wv_c,
                "wo": wo_c,
                "cs2": cs2,
                "sn2": sn2,
                "mk": mk,
            }
        )
    return in_maps


_CACHE = {}


def kernel_with_results(x, token_positions, qkv_w, o_w, trace=False, **kw):
    from concourse.bass_utils import run_bass_kernel_spmd

    if "nc" not in _CACHE:
        _CACHE["nc"] = build_module()
    nc = _CACHE["nc"]

    in_maps = prepare_in_maps(x, token_positions, qkv_w, o_w)
    res = run_bass_kernel_spmd(
        nc, in_maps, core_ids=list(range(NCORES)), trace=trace, **kw
    )
    acc = np.zeros((B * S, D_MODEL), dtype=np.float64)
    for r in res.results:
        acc += r["y"].astype(np.float64)
    return acc.astype(np.float32).reshape(B, S, D_MODEL), res


def kernel(x, token_positions, qkv_w, o_w):
    out, _ = kernel_with_results(x, token_positions, qkv_w, o_w)
    return out


# revision 46
# speedup vs baseline: 1.0289x; 1.0049x over previous
"""Causal multi-head attention with RoPE on 8 Trainium2 NeuronCores.

Tensor-parallel over heads: each core owns 2 of the 16 heads (its slice of
qkv_w rows and o_w columns), computes attention + its partial o_proj for
both batch elements, and the host sums the 8 partial outputs (the
"all-reduce").

Device layout choices (see module builder):
  - q/k computed feature-major [dk, tokens] so scores need no transposes
  - scores computed transposed [k, q]; softmax denominator = ones-matmul
    partition reduction; no max-subtraction (scores are bounded, exp is
    safe in fp32)
  - RoPE via even/odd row-permuted projection weights + a DMA partition
    half-swap + 3 full-width vector ops
  - v kept token-major so attn@v consumes exp(scores) directly
  - all big matmuls in bf16 (1 cycle/row); accumulation in fp32 PSUM
"""

import math
from contextlib import ExitStack

import numpy as np
import ml_dtypes

D_MODEL = 2048
NUM_HEADS = 16
HEAD_DIM = 128
THETA = 10000.0
B = 2
S = 2048
NCORES = 8
HPC = NUM_HEADS // NCORES  # heads per core
F = HPC * HEAD_DIM  # q/k/v features per core

BF16 = ml_dtypes.bfloat16


def build_module(D=D_MODEL, S_=S, B_=B):
    import concourse.mybir as mybir
    import concourse.tile as tile
    from concourse import bacc
    from concourse.bass import ts, ds

    f32 = mybir.dt.float32
    bf16 = mybir.dt.bfloat16
    f32r = mybir.dt.float32r
    Exp = mybir.ActivationFunctionType.Exp

    T = B_ * S_
    DC = D // 128  # contraction chunks
    TG = 512  # token group for qkv projection
    NTG = T // TG
    GPB = S_ // TG  # t-groups per batch
    NQG = S_ // 512  # q-groups per batch
    NVC = T // 128  # v token chunks
    KPB = S_ // 128  # k tiles per batch
    scale = 1.0 / math.sqrt(HEAD_DIM)

    nc = bacc.Bacc("TRN2", target_bir_lowering=False, debug=False)

    xt = nc.dram_tensor("xt", [D, T], bf16, kind="ExternalInput")[:]
    wq = nc.dram_tensor("wq", [D, F], bf16, kind="ExternalInput")[:]
    wk = nc.dram_tensor("wk", [D, F], bf16, kind="ExternalInput")[:]
    wv = nc.dram_tensor("wv", [D, F], bf16, kind="ExternalInput")[:]
    wo = nc.dram_tensor("wo", [F, D], bf16, kind="ExternalInput")[:]
    cs2 = nc.dram_tensor("cs2", [128, S_], f32, kind="ExternalInput")[:]
    sn2 = nc.dram_tensor("sn2", [128, S_], f32, kind="ExternalInput")[:]
    mk = nc.dram_tensor("mk", [128, 4 * 512], bf16, kind="ExternalInput")[:]
    y = nc.dram_tensor("y", [T, D], f32, kind="ExternalOutput")[:]

    with tile.TileContext(nc) as tc:
        with ExitStack() as ctx:
            const = ctx.enter_context(tc.tile_pool(name="const", bufs=1))
            xpool = ctx.enter_context(tc.tile_pool(name="xpool", bufs=2))
            store = ctx.enter_context(tc.tile_pool(name="store", bufs=1))
            rope = ctx.enter_context(tc.tile_pool(name="rope", bufs=2))
            epool = ctx.enter_context(tc.tile_pool(name="epool", bufs=6))
            spool = ctx.enter_context(tc.tile_pool(name="spool", bufs=2))
            psum = ctx.enter_context(tc.tile_pool(name="psum", bufs=3, space="PSUM"))
            psgen = ctx.enter_context(tc.tile_pool(name="psgen", bufs=2, space="PSUM"))
            psacc = ctx.enter_context(tc.tile_pool(name="psacc", bufs=2, space="PSUM"))
            psred = ctx.enter_context(tc.tile_pool(name="psred", bufs=1, space="PSUM"))

            # ---- constants ----
            # DMA order matters at startup: the first QKV f-tile only needs
            # wq + xg(0); put those first on the queue. xg(0) itself is
            # emitted by the first emit_xg call below.
            wq_sb = const.tile([128, DC, F], bf16, tag="wq")
            nc.sync.dma_start(out=wq_sb[:], in_=wq.rearrange("(o p) f -> p o f", p=128))
            wk_sb = const.tile([128, DC, F], bf16, tag="wk")
            wv_sb = const.tile([128, DC, F], bf16, tag="wv")
            wo_sb = const.tile([128, HPC, D], bf16, tag="wo")
            cs2_sb = const.tile([128, S_], f32, tag="cs2")
            sn2_sb = const.tile([128, S_], f32, tag="sn2")
            mk_sb = const.tile([128, 4, 512], bf16, tag="mk")
            ones_sb = const.tile([128, 128], bf16, tag="ones")

            def emit_late_consts():
                # rope tables for the first token block only, then the rest
                # wk/wv ride the otherwise-idle scalar HWDGE ring so they
                # land in parallel with the sync ring's wq/xg/table loads
                nc.scalar.dma_start(
                    out=wk_sb[:], in_=wk.rearrange("(o p) f -> p o f", p=128)
                )
                nc.scalar.dma_start(
                    out=wv_sb[:], in_=wv.rearrange("(o p) f -> p o f", p=128)
                )
                nc.sync.dma_start(out=cs2_sb[:, 0:TG], in_=cs2[:, 0:TG])
                nc.sync.dma_start(out=sn2_sb[:, 0:TG], in_=sn2[:, 0:TG])
                if GPB > 1:
                    emit_xg(1)
                    nc.sync.dma_start(out=cs2_sb[:, TG:], in_=cs2[:, TG:])
                    nc.sync.dma_start(out=sn2_sb[:, TG:], in_=sn2[:, TG:])
                nc.vector.memset(ones_sb[:], 1.0)

            def emit_attn_consts():
                nc.sync.dma_start(
                    out=mk_sb[:], in_=mk.rearrange("p (m q) -> p m q", m=4)
                )
                nc.sync.dma_start(
                    out=wo_sb[:], in_=wo.rearrange("(o p) e -> p o e", p=128)
                )

            # ---- persistent stores ----
            q_sb = store.tile([128, HPC, T], bf16, tag="q")  # [dk, ht, tok]
            k_sb = store.tile([128, HPC, T], bf16, tag="k")
            v_sb = store.tile([128, NVC, F], bf16, tag="v")  # [tok128, chunk, f]
            ao_sb = store.tile([128, HPC, T], bf16, tag="ao")  # [dk, ht, tok]

            xt_r = xt.rearrange("(o p) t -> p o t", p=128)

            qk_jobs = [(wq_sb, q_sb, 0), (wq_sb, q_sb, 1), (wk_sb, k_sb, 0), (wk_sb, k_sb, 1)]
            xg_tiles = {}

            def emit_xg(g, split=1):
                xg = xpool.tile([128, DC, TG], bf16, tag="xg", name=f"xg_{g}")
                # split lets the first matmul start after the first chunk lands
                step = DC // split
                for s in range(split):
                    nc.sync.dma_start(
                        out=xg[:, ts(s, step), :],
                        in_=xt_r[:, ts(s, step), ts(g, TG)],
                    )
                xg_tiles[g] = xg

            def emit_qk_ftile(g, job):
                w_sb, qk_store, ht = job
                xg = xg_tiles[g]
                pos_sl = ds((g % GPB) * TG, TG)
                ps = psgen.tile([128, TG], f32, tag="gen")
                for dc in range(DC):
                    nc.tensor.matmul(
                        ps[:],
                        lhsT=w_sb[:, dc, ts(ht, 128)],
                        rhs=xg[:, dc, :],
                        start=(dc == 0),
                        stop=(dc == DC - 1),
                    )
                # rope: rot = raw*cos2 + halfswap(raw*sin2_pre)
                t0 = rope.tile([128, TG], f32, tag="t0")
                nc.vector.tensor_mul(out=t0[:], in0=ps[:], in1=cs2_sb[:, pos_sl])
                t1s = rope.tile([128, TG], f32, tag="t1s")
                nc.vector.tensor_mul(out=t1s[:], in0=ps[:], in1=sn2_sb[:, pos_sl])
                # scalar (ACT) HWDGE ring: keeps these latency-critical swaps
                # out from behind the 2MB xg loads on the sync ring
                t1w = rope.tile([128, TG], f32, tag="t1w")
                nc.scalar.dma_start(out=t1w[0:64, :], in_=t1s[64:128, :])
                nc.scalar.dma_start(out=t1w[64:128, :], in_=t1s[0:64, :])
                nc.vector.tensor_add(
                    out=qk_store[:, ht, ts(g, TG)], in0=t0[:], in1=t1w[:]
                )

            def emit_v_sub(g, tsub):
                xg = xg_tiles[g]
                psv = psgen.tile([128, F], f32, tag="gen")
                for dc in range(DC):
                    nc.tensor.matmul(
                        psv[:],
                        lhsT=xg[:, dc, ts(tsub, 128)],
                        rhs=wv_sb[:, dc, :],
                        start=(dc == 0),
                        stop=(dc == DC - 1),
                    )
                nc.scalar.copy(out=v_sb[:, g * (TG // 128) + tsub, :], in_=psv[:])

            def emit_qkv_group(g):
                emit_xg(g)
                for job in qk_jobs:
                    emit_qk_ftile(g, job)
                for tsub in range(TG // 128):
                    emit_v_sub(g, tsub)

            def emit_oproj_eg(tt, eg, ystate):
                if eg == 0:
                    ystate[tt] = rope.tile([128, D], f32, tag="ysb", name=f"ysb_{tt}")
                y_sb = ystate[tt]
                yp = psgen.tile([128, 512], f32, tag="gen")
                for ht in range(HPC):
                    nc.tensor.matmul(
                        yp[:],
                        lhsT=ao_sb[:, ht, ts(tt, 128)],
                        rhs=wo_sb[:, ht, ts(eg, 512)],
                        start=(ht == 0),
                        stop=(ht == HPC - 1),
                    )
                if eg % 2 == 0:
                    nc.vector.tensor_copy(out=y_sb[:, ts(eg, 512)], in_=yp[:])
                else:
                    nc.scalar.copy(out=y_sb[:, ts(eg, 512)], in_=yp[:])
                if eg == D // 512 - 1:
                    nc.sync.dma_start(out=y[ts(tt, 128), :], in_=y_sb[:])
                    del ystate[tt]

            ystate = {}

            def emit_oproj_tile(tt):
                for eg in range(D // 512):
                    emit_oproj_eg(tt, eg, ystate)

            # filler queue: zero-dependency emission units pumped into PE
            # stalls of the attention dependency chain
            from collections import deque

            filler = deque()

            def pump(n=1):
                for _ in range(n):
                    if filler:
                        filler.popleft()()

            def drain():
                while filler:
                    filler.popleft()()

            LOOKAHEAD = 2

            def emit_attention_batch(b, post_group_hook=None):
                pending = [None]
                total_steps = HPC * sum(4 * m + 4 for m in range(NQG))
                state = {"done": 0, "acc": 0.0}

                def paced_pump():
                    # spread the filler queue evenly over the remaining
                    # attention steps so PE never runs a bare chain
                    state["done"] += 1
                    remaining = max(1, total_steps - state["done"])
                    state["acc"] += len(filler) / remaining
                    n = int(state["acc"])
                    if n:
                        state["acc"] -= n
                        pump(n)

                def do_pending():
                    if pending[0] is not None:
                        pending[0]()
                        pending[0] = None

                for m in range(NQG):
                    for ht in range(HPC):
                        qv = q_sb[:, ht, ds(b * S_, S_)]
                        kv = k_sb[:, ht, ds(b * S_, S_)]
                        nk = 4 * m + 4
                        ao = psacc.tile([128, 512], f32, tag="acc")
                        sacc = spool.tile([128, 512], f32, tag="sacc")
                        eTs = {}
                        steps = 0

                        def emit_scores(j, m=m, ao=ao, sacc=sacc, eTs=eTs, qv=qv, kv=kv):
                            p = j - 4 * m  # >= 0 on diagonal superblock
                            col0 = max(0, p * 128)
                            ncol = 512 - col0
                            s_ps = psum.tile([128, 512], f32, tag="work")
                            nc.tensor.matmul(
                                s_ps[:, col0:],
                                lhsT=kv[:, ts(j, 128)],
                                rhs=qv[:, ds(m * 512 + col0, ncol)],
                                start=True,
                                stop=True,
                            )
                            eT = epool.tile([128, 512], bf16, tag="eT")
                            nc.scalar.activation(
                                out=eT[:, col0:], in_=s_ps[:, col0:], func=Exp, scale=scale
                            )
                            if p >= 0:
                                nc.vector.tensor_mul(
                                    out=eT[:, col0:],
                                    in0=eT[:, col0:],
                                    in1=mk_sb[:, p, col0:],
                                )
                            if j == 0:
                                nc.vector.tensor_copy(out=sacc[:], in_=eT[:])
                            else:
                                nc.vector.tensor_add(
                                    out=sacc[:, col0:],
                                    in0=sacc[:, col0:],
                                    in1=eT[:, col0:],
                                )
                            eTs[j] = (eT, col0)

                        def emit_attnv(j, m=m, ao=ao, eTs=eTs, b=b, ht=ht, nk=nk):
                            eT, col0 = eTs.pop(j)
                            nc.tensor.matmul(
                                ao[:, col0:],
                                lhsT=v_sb[:, b * KPB + j, ts(ht, 128)],
                                rhs=eT[:, col0:],
                                start=(j == 0),
                                stop=(j == nk - 1),
                            )

                        for j in range(nk):
                            emit_scores(j)
                            if j == 3:
                                # past the diagonal-heavy group start, so the
                                # finalize's DVE burst doesn't delay exp/mask
                                do_pending()
                            if j >= LOOKAHEAD:
                                emit_attnv(j - LOOKAHEAD)
                            steps += 1
                            paced_pump()
                        for j in range(max(0, nk - LOOKAHEAD), nk):
                            emit_attnv(j)

                        def finalize(b=b, ht=ht, m=m, ao=ao, sacc=sacc):
                            sacc_b = epool.tile([128, 512], bf16, tag="saccb")
                            nc.vector.tensor_copy(out=sacc_b[:], in_=sacc[:])
                            rps = psred.tile([128, 512], f32, tag="red")
                            nc.tensor.matmul(
                                rps[:],
                                lhsT=ones_sb[:],
                                rhs=sacc_b[:],
                                start=True,
                                stop=True,
                            )
                            rsb = spool.tile([128, 512], f32, tag="rsb")
                            nc.vector.reciprocal_approx_fast(out=rsb[:], in_=rps[:])
                            nc.vector.tensor_mul(
                                out=ao_sb[:, ht, ds(b * S_ + m * 512, 512)],
                                in0=ao[:],
                                in1=rsb[:],
                            )
                            if ht == HPC - 1 and post_group_hook is not None:
                                post_group_hook(m)

                        pending[0] = finalize
                do_pending()

            # ---- program order ----
            emit_xg(0, split=4)
            emit_late_consts()
            for g in range(GPB):  # batch-0 qkv, dense; prefetch next xg
                if 1 < g + 1 < GPB:
                    emit_xg(g + 1)
                for job in qk_jobs:
                    emit_qk_ftile(g, job)
                for tsub in range(TG // 128):
                    emit_v_sub(g, tsub)
            emit_attn_consts()
            if B_ > 1:
                for g in range(GPB, NTG):  # batch-1 qkv as filler
                    filler.append((lambda g=g: emit_xg(g)))
                    for job in qk_jobs:
                        filler.append(lambda g=g, job=job: emit_qk_ftile(g, job))
                    for tsub in range(TG // 128):
                        filler.append(lambda g=g, tsub=tsub: emit_v_sub(g, tsub))

            def oproj_hook(b):
                def hook(m):
                    # q-group m fully normalized -> its four 128-token oproj
                    # tiles are ready; queue them as filler
                    base = b * (T // 256)
                    for tt in range(base + 4 * m, base + 4 * m + 4):
                        filler.append(lambda tt=tt: emit_oproj_tile(tt))

                return hook

            emit_attention_batch(0, post_group_hook=oproj_hook(0) if B_ > 1 else None)
            drain()  # leftovers of batch-1 qkv / batch-0 oproj
            if B_ > 1:
                emit_attention_batch(1, post_group_hook=oproj_hook(1))
                drain()
            else:
                for tt in range(T // 128):
                    emit_oproj_tile(tt)

    nc.compile()
    return nc


def _rope_tables(token_positions, S_):
    pos = np.asarray(token_positions).astype(np.float32)
    dim_id = np.arange(0, HEAD_DIM, 2, dtype=np.float32)
    inv_freq = np.power(np.float32(THETA), dim_id / np.float32(HEAD_DIM)).astype(
        np.float32
    )
    ang = (pos[None, :] / inv_freq[:, None]).astype(np.float32)  # [64, S]
    cos = np.cos(ang).astype(np.float32)
    sin = np.sin(ang).astype(np.float32)
    cs2 = np.concatenate([cos, cos], axis=0)  # [128, S]
    # pre-multiply sign layout: top half (x0 rows) gets +sin (feeds r1 after
    # the half-swap), bottom half (x1 rows) gets -sin (feeds r0)
    sn2 = np.concatenate([sin, -sin], axis=0)
    return np.ascontiguousarray(cs2), np.ascontiguousarray(sn2)


def _masks():
    mk = np.zeros((128, 4, 512), dtype=np.float32)
    kl = np.arange(128)[:, None]
    ql = np.arange(512)[None, :]
    for p in range(4):
        mk[:, p, :] = ((p * 128 + kl) <= ql).astype(np.float32)
    return np.ascontiguousarray(mk.reshape(128, 4 * 512).astype(BF16))


def _perm(n_heads):
    # within each 128-feature head block: evens then odds
    p = []
    for h in range(n_heads):
        base = h * HEAD_DIM
        p.extend(range(base, base + HEAD_DIM, 2))
        p.extend(range(base + 1, base + HEAD_DIM, 2))
    return np.array(p, dtype=np.int64)


def prepare_in_maps(x, token_positions, qkv_w, o_w, D=D_MODEL, S_=S, B_=B, ncores=NCORES):
    T = B_ * S_
    x = np.asarray(x, dtype=np.float32)
    qkv_w = np.asarray(qkv_w, dtype=np.float32)
    o_w = np.asarray(o_w, dtype=np.float32)

    xt = np.ascontiguousarray(x.reshape(T, D).T).astype(BF16)  # [D, T]
    cs2, sn2 = _rope_tables(token_positions, S_)
    mk = _masks()
    perm = _perm(HPC)

    in_maps = []
    for c in range(ncores):
        r0 = c * F
        qrows = qkv_w[r0 : r0 + F]
        krows = qkv_w[D + r0 : D + r0 + F]
        vrows = qkv_w[2 * D + r0 : 2 * D + r0 + F]
        wq_c = np.ascontiguousarray(qrows[perm].T).astype(BF16)  # [D, F]
        wk_c = np.ascontiguousarray(krows[perm].T).astype(BF16)
        wv_c = np.ascontiguousarray(vrows.T).astype(BF16)
        wo_c = np.ascontiguousarray(o_w[:, r0 : r0 + F].T).astype(BF16)  # [F, D]
        in_maps.append(
            {
                "xt": xt,
                "wq": wq_c,
                "wk": wk_c,
                "wv": wv_c,
                "wo": wo_c,
                "cs2": cs2,
                "sn2": sn2,
                "mk": mk,
            }
        )
    return in_maps


_CACHE = {}


def kernel_with_results(x, token_positions, qkv_w, o_w, trace=False, **kw):
    from concourse.bass_utils import run_bass_kernel_spmd

    if "nc" not in _CACHE:
        _CACHE["nc"] = build_module()
    nc = _CACHE["nc"]

    in_maps = prepare_in_maps(x, token_positions, qkv_w, o_w)
    res = run_bass_kernel_spmd(
        nc, in_maps, core_ids=list(range(NCORES)), trace=trace, **kw
    )
    acc = np.zeros((B * S, D_MODEL), dtype=np.float64)
    for r in res.results:
        acc += r["y"].astype(np.float64)
    return acc.astype(np.float32).reshape(B, S, D_MODEL), res


def kernel(x, token_positions, qkv_w, o_w):
    out, _ = kernel_with_results(x, token_positions, qkv_w, o_w)
    return out
